# revision 7
# baseline (speedup 1.0000x reference)
"""GCN message-passing kernel for 8 TRN2 NeuronCores.

Strategy (graph/data parallel, dst-sharded):
  - Nodes sharded 6250/core. Per layer: each core computes its shard of the
    gather table H' (dense matmul + epilogue), an AllGather builds the full
    50000x128 fp16 table in HBM.
  - Edges (incl. self-loops) sorted by dst, grouped into 128-wide dst blocks.
    Per 128-edge tile: dma_gather fetches H'[src] rows (256B fp16), DVE
    is_equal builds the one-hot [edge, dst_local] tile, PE accumulates
    Onehot^T @ G into the block's PSUM (segment sum).
  - GCN norm dinv[src]*dinv[dst] is separable: src side folded into the
    table rows, dst side into the block epilogue. BN folded into W and B.
  - Epilogues: dense1: tbl = dinv*(x@W1A); gather1/2: z = dinv*relu(S*dinv+B)
    (the outer dinv pre-applies the next table's src factor); dense2/3 are
    then plain copies; gather3: y3 = relu(S*dinv+B) feeds the MLP head.
  - 4-layer MLP runs feature-major per core; output [8, 6272] f32 is
    transposed/trimmed on the host.
"""

import sys

sys.path.insert(0, "/opt/trn_rl_repo")

import numpy as np

import concourse.bacc as bacc
import concourse.mybir as mybir
import concourse.tile as tile
from concourse.bass_utils import run_bass_kernel_spmd

# Problem constants
N, E, IN, HC = 50000, 800000, 64, 96
FC1, FC2, FC3, OUT = 256, 128, 64, 8
EPS = 1e-5
NCORES = 8
NPER = N // NCORES          # 6250 nodes per core
BW = 128                    # dst-block width
NB = (NPER + BW - 1) // BW  # 49 blocks
NPAD = NB * BW              # 6272
SPLIT = 25000               # gather-table split (int16 idx limit)
TC = 8                      # tiles per gather chunk (1024 idxs; 4096 crashed HW)


def _set_dims(n, e, split=None, tc=None):
    """Test hook: shrink the problem (keeps feature dims)."""
    global N, E, NPER, NB, NPAD, SPLIT, TC
    N, E = n, e
    NPER = N // NCORES
    NB = (NPER + BW - 1) // BW
    NPAD = NB * BW
    SPLIT = split if split is not None else (N // 2 // 8 * 8)
    if tc is not None:
        TC = tc

F16 = mybir.dt.float16
F32 = mybir.dt.float32
I16 = mybir.dt.int16

Alu = mybir.AluOpType
Act = mybir.ActivationFunctionType


def _wrap_idx(idx_stream):
    """int16 idx stream -> [128, len/16] wrapped layout (pos i -> [i%16, i//16]),
    replicated 8x across partition groups."""
    n = idx_stream.shape[0]
    assert n % 16 == 0
    a = idx_stream.reshape(n // 16, 16).T.astype(np.int16)
    return np.ascontiguousarray(np.tile(a, (8, 1)))


def _preprocess(inputs):
    """Host-side graph preprocessing. Returns (structure, per-core in_maps)."""
    x = np.asarray(inputs["x"], np.float32)
    edge_index = np.asarray(inputs["edge_index"], np.int64)
    src = np.concatenate([edge_index[0], np.arange(N, dtype=np.int64)])
    dst = np.concatenate([edge_index[1], np.arange(N, dtype=np.int64)])
    deg = np.bincount(dst, minlength=N).astype(np.float32)
    dinv = (1.0 / np.sqrt(deg)).astype(np.float32)

    order = np.argsort(dst, kind="stable")
    src_s = src[order]
    dst_s = dst[order]

    core_edge_start = np.searchsorted(dst_s, np.arange(0, N + 1, NPER))
    per_core = []
    nlo = np.zeros((NCORES, NB), dtype=np.int64)
    nhi = np.zeros((NCORES, NB), dtype=np.int64)
    for c in range(NCORES):
        s0, s1 = core_edge_start[c], core_edge_start[c + 1]
        cs, cd = src_s[s0:s1], dst_s[s0:s1] - c * NPER
        blk = cd // BW
        lo = cs < SPLIT
        lists = []
        for b in range(NB):
            m = blk == b
            mlo = m & lo
            mhi = m & ~lo
            lists.append((cs[mlo], cd[mlo], cs[mhi], cd[mhi]))
            nlo[c, b] = int(mlo.sum())
            nhi[c, b] = int(mhi.sum())
        per_core.append(lists)

    Tlo = np.ceil(nlo.max(axis=0) / 128).astype(int)
    Thi = np.ceil(nhi.max(axis=0) / 128).astype(int)
    TLO, THI = int(Tlo.sum()), int(Thi.sum())
    lo_t0 = np.concatenate([[0], np.cumsum(Tlo)])[:-1]
    hi_t0 = np.concatenate([[0], np.cumsum(Thi)])[:-1]

    structure = dict(Tlo=Tlo.tolist(), Thi=Thi.tolist(), TLO=TLO, THI=THI,
                     lo_t0=lo_t0.tolist(), hi_t0=hi_t0.tolist())

    k = 1.0 / np.sqrt(1.0 + EPS)

    def fold(w, b, g, be):
        A = (np.asarray(g, np.float32) * k)
        Wp = (np.asarray(w, np.float32) * A[None, :]).astype(np.float16)
        B = (np.asarray(b, np.float32) * A + np.asarray(be, np.float32))
        return Wp, np.tile(B[None, :].astype(np.float32), (128, 1))

    w1p, b1rep = fold(inputs["w1"], inputs["b1"], inputs["g1"], inputs["be1"])
    w2p, b2rep = fold(inputs["w2"], inputs["b2"], inputs["g2"], inputs["be2"])
    w3p, b3rep = fold(inputs["w3"], inputs["b3"], inputs["g3"], inputs["be3"])

    lw1 = np.asarray(inputs["lw1"], np.float32).astype(np.float16)
    lw2 = np.asarray(inputs["lw2"], np.float32).astype(np.float16)
    lw3 = np.asarray(inputs["lw3"], np.float32).astype(np.float16)
    lw4 = np.asarray(inputs["lw4"], np.float32).astype(np.float16)

    shared = {
        "w1p": w1p, "w2p": w2p, "w3p": w3p,
        "b1rep": b1rep, "b2rep": b2rep, "b3rep": b3rep,
        "lw1a": np.ascontiguousarray(lw1[:, :128]),
        "lw1b": np.ascontiguousarray(lw1[:, 128:]),
        "lw2a": np.ascontiguousarray(lw2[:128, :]),
        "lw2b": np.ascontiguousarray(lw2[128:, :]),
        "lw3": lw3, "lw4": lw4,
        "lb1a": np.asarray(inputs["lb1"], np.float32)[:128, None].copy(),
        "lb1b": np.asarray(inputs["lb1"], np.float32)[128:, None].copy(),
        "lb2": np.asarray(inputs["lb2"], np.float32)[:, None].copy(),
        "lb3": np.asarray(inputs["lb3"], np.float32)[:, None].copy(),
        "lb4": np.asarray(inputs["lb4"], np.float32)[:, None].copy(),
        "iota": np.ascontiguousarray(
            np.tile(np.arange(128, dtype=np.float16)[None, :], (128, 1))),
    }

    in_maps = []
    for c in range(NCORES):
        idx_lo = np.zeros(max(TLO, 1) * 128, dtype=np.int16)
        dl_lo = np.full(max(TLO, 1) * 128, -1.0, dtype=np.float32)
        idx_hi = np.zeros(max(THI, 1) * 128, dtype=np.int16)
        dl_hi = np.full(max(THI, 1) * 128, -1.0, dtype=np.float32)
        for b in range(NB):
            cs_lo, cd_lo, cs_hi, cd_hi = per_core[c][b]
            o = lo_t0[b] * 128
            idx_lo[o:o + len(cs_lo)] = cs_lo.astype(np.int16)
            dl_lo[o:o + len(cd_lo)] = (cd_lo - b * BW).astype(np.float32)
            o = hi_t0[b] * 128
            idx_hi[o:o + len(cs_hi)] = (cs_hi - SPLIT).astype(np.int16)
            dl_hi[o:o + len(cd_hi)] = (cd_hi - b * BW).astype(np.float32)

        dv = np.ones(NPAD, dtype=np.float32)
        dv[:NPER] = dinv[c * NPER:(c + 1) * NPER]
        dinv_loc = np.ascontiguousarray(dv.reshape(NB, 128).T)

        xT = np.zeros((IN, NPAD), dtype=np.float16)
        xT[:, :NPER] = x[c * NPER:(c + 1) * NPER].T.astype(np.float16)

        m = dict(shared)
        m.update({
            "xT": xT,
            "dinv": dinv_loc,
            "idxlo": _wrap_idx(idx_lo),
            "idxhi": _wrap_idx(idx_hi),
            "dllo": np.ascontiguousarray(dl_lo.reshape(-1, 128).T),
            "dlhi": np.ascontiguousarray(dl_hi.reshape(-1, 128).T),
        })
        in_maps.append(m)

    return structure, in_maps


def _build(structure):
    """Build the SPMD Bass graph (shared by all 8 cores)."""
    Tlo, Thi = structure["Tlo"], structure["Thi"]
    TLO, THI = structure["TLO"], structure["THI"]
    lo_t0, hi_t0 = structure["lo_t0"], structure["hi_t0"]
    TLOp, THIp = max(TLO, 1), max(THI, 1)
    cores = list(range(NCORES))

    nc = bacc.Bacc("TRN2", target_bir_lowering=False, debug=False,
                   num_devices=NCORES)

    P = {}
    def par(name, shape, dtype, out=False):
        P[name] = nc.declare_dram_parameter(name, shape, dtype, isOutput=out)
        return P[name]

    par("xT", [IN, NPAD], F16)
    par("w1p", [IN, HC], F16); par("w2p", [HC, HC], F16); par("w3p", [HC, HC], F16)
    par("b1rep", [128, HC], F32); par("b2rep", [128, HC], F32); par("b3rep", [128, HC], F32)
    par("dinv", [128, NB], F32)
    par("iota", [128, 128], F16)
    par("idxlo", [128, 8 * TLOp], I16); par("idxhi", [128, 8 * THIp], I16)
    par("dllo", [128, TLOp], F32); par("dlhi", [128, THIp], F32)
    par("lw1a", [HC, 128], F16); par("lw1b", [HC, FC1 - 128], F16)
    par("lw2a", [128, FC2], F16); par("lw2b", [FC1 - 128, FC2], F16)
    par("lw3", [FC2, FC3], F16); par("lw4", [FC3, OUT], F16)
    par("lb1a", [128, 1], F32); par("lb1b", [FC1 - 128, 1], F32)
    par("lb2", [FC2, 1], F32); par("lb3", [FC3, 1], F32); par("lb4", [OUT, 1], F32)
    par("out", [OUT, NPAD], F32, out=True)

    with tile.TileContext(nc) as tc:
        with (
            tc.tile_pool(name="const", bufs=1) as cpool,
            tc.tile_pool(name="dram", bufs=1, space="DRAM") as dpool,
            tc.tile_pool(name="hp", bufs=2) as hp_pool,
            tc.tile_pool(name="zt", bufs=2) as zt_pool,
        ):
            C = {}
            for name, p in P.items():
                if name == "out":
                    continue
                t = cpool.tile(list(p.shape), p.dtype, name=f"c_{name}")
                nc.sync.dma_start(t[:], p[:])
                C[name] = t

            agin = [dpool.tile([NPAD, 128], F16, name=f"agin{l}") for l in range(3)]
            hfull = [dpool.tile([N, 128], F16, addr_space="Shared",
                                name=f"hfull{l}") for l in range(3)]
            zscr = [dpool.tile([NPAD, 128], F16, name=f"zscr{l}") for l in range(3)]

            w_sb = [C["w1p"], C["w2p"], C["w3p"]]
            b_sb = [C["b1rep"], C["b2rep"], C["b3rep"]]
            dinv_sb, iota_sb = C["dinv"], C["iota"]

            def big_alloc(name):
                """[128, NPAD] fp16 big tile with pad cols (96:128) zeroed."""
                t = hp_pool.tile([128, NPAD], F16, tag="hp", name=name)
                nc.vector.memset(
                    t[:].rearrange("p (t f) -> p t f", f=128)[:, :, HC:128], 0.0)
                return t

            def big_to_rows(dram_t, big_sb):
                # SBUF [p, t*128+f] -> HBM rows [t*128+p, f]
                nc.sync.dma_start(
                    dram_t[:].rearrange("(t p) f -> p t f", p=128),
                    big_sb[:].rearrange("p (t f) -> p t f", f=128))

            def allgather(l):
                nc.gpsimd.collective_compute(
                    "AllGather", Alu.bypass, replica_groups=[cores],
                    ins=[agin[l][0:NPER, :]], outs=[hfull[l][:]])

            def dense(l, zt, fin, psd_pool, scale_dinv):
                """hp_big = (Z @ Wl') [* dinv]; returns the big SBUF tile."""
                hp = big_alloc(f"hpd{l}")
                for t in range(NB):
                    ps = psd_pool.tile([128, HC], F32, tag="psd")
                    nc.tensor.matmul(ps[:], zt[0:fin, t * 128:(t + 1) * 128],
                                     w_sb[l][:], start=True, stop=True)
                    if scale_dinv:
                        nc.vector.tensor_scalar(
                            hp[:, t * 128:t * 128 + HC], ps[:],
                            dinv_sb[:, t:t + 1], None, Alu.mult)
                    else:
                        nc.vector.tensor_copy(hp[:, t * 128:t * 128 + HC], ps[:])
                return hp

            def gather_stage(l, premult, pools):
                """Aggregate edges against hfull[l] -> big SBUF tile."""
                glo_pool, ghi_pool, ind_pool, tmp_pool, psb_pool = pools
                out_big = big_alloc(f"agg{l}")
                hf = hfull[l]
                streams = {
                    "lo": dict(T=TLO, t0=lo_t0, nt=Tlo, dl=C["dllo"],
                               idx=C["idxlo"], src=hf[0:SPLIT, :],
                               pool=glo_pool, cache={}),
                    "hi": dict(T=THI, t0=hi_t0, nt=Thi, dl=C["dlhi"],
                               idx=C["idxhi"], src=hf[SPLIT:N, :],
                               pool=ghi_pool, cache={}),
                }

                def fetch(s, sname, k):
                    if k in s["cache"]:
                        return s["cache"][k]
                    t0 = k * TC
                    ct = min(TC, s["T"] - t0)
                    g = s["pool"].tile([128, TC * 128], F16, tag=f"g{sname}",
                                       name=f"g{sname}_{l}_{k}")
                    nidx = ct * 128
                    nc.gpsimd.dma_gather(
                        g[:, 0:nidx].rearrange("p (t e) -> p t e", e=128),
                        s["src"],
                        s["idx"][:, t0 * 8:(t0 + ct) * 8],
                        nidx, nidx, 128)
                    s["cache"][k] = g
                    return g

                for b in range(NB):
                    ntiles = Tlo[b] + Thi[b]
                    ps = psb_pool.tile([128, HC], F32, tag="psb",
                                       name=f"psb{l}_{b}")
                    i = 0
                    for sname in ("lo", "hi"):
                        s = streams[sname]
                        for j in range(s["nt"][b]):
                            t = s["t0"][b] + j
                            k = t // TC
                            g = fetch(s, sname, k)
                            ind = ind_pool.tile([128, 128], F16, tag="ind")
                            nc.vector.tensor_scalar(
                                ind[:], iota_sb[:], s["dl"][:, t:t + 1], None,
                                Alu.is_equal)
                            jj = t - k * TC
                            nc.tensor.matmul(
                                ps[:], ind[:], g[:, jj * 128:jj * 128 + HC],
                                start=(i == 0), stop=(i == ntiles - 1))
                            i += 1
                    tmp = tmp_pool.tile([128, HC], F32, tag="tmp")
                    nc.vector.scalar_tensor_tensor(
                        tmp[:], ps[:], dinv_sb[:, b:b + 1], b_sb[l][:],
                        Alu.mult, Alu.add)
                    if premult:
                        nc.scalar.activation(
                            out_big[:, b * 128:b * 128 + HC], tmp[:], Act.Relu,
                            scale=dinv_sb[:, b:b + 1])
                    else:
                        nc.scalar.activation(
                            out_big[:, b * 128:b * 128 + HC], tmp[:], Act.Relu)
                return out_big

            with (
                tc.tile_pool(name="glo", bufs=3) as glo_pool,
                tc.tile_pool(name="ghi", bufs=3) as ghi_pool,
                tc.tile_pool(name="ind", bufs=6) as ind_pool,
                tc.tile_pool(name="tmp", bufs=4) as tmp_pool,
                tc.tile_pool(name="psd", bufs=2, space="PSUM") as psd_pool,
                tc.tile_pool(name="psb", bufs=4, space="PSUM") as psb_pool,
            ):
                gpools = (glo_pool, ghi_pool, ind_pool, tmp_pool, psb_pool)

                # layer 1
                hp1 = dense(0, C["xT"], IN, psd_pool, scale_dinv=True)
                big_to_rows(agin[0], hp1)
                allgather(0)
                z2 = gather_stage(0, premult=True, pools=gpools)

                # layer 2
                big_to_rows(zscr[0], z2)
                zt2 = zt_pool.tile([128, NPAD], F16, tag="zt", name="zt2")
                nc.sync.dma_start_transpose(zt2[:], zscr[0][:])
                hp2 = dense(1, zt2, HC, psd_pool, scale_dinv=False)
                big_to_rows(agin[1], hp2)
                allgather(1)
                z3 = gather_stage(1, premult=True, pools=gpools)

                # layer 3
                big_to_rows(zscr[1], z3)
                zt3 = zt_pool.tile([128, NPAD], F16, tag="zt", name="zt3")
                nc.sync.dma_start_transpose(zt3[:], zscr[1][:])
                hp3 = dense(2, zt3, HC, psd_pool, scale_dinv=False)
                big_to_rows(agin[2], hp3)
                allgather(2)
                y3 = gather_stage(2, premult=False, pools=gpools)
                big_to_rows(zscr[2], y3)

            # MLP head (feature-major), own PSUM pool
            with (
                tc.tile_pool(name="psm", bufs=8, space="PSUM") as psm_pool,
                tc.tile_pool(name="mz", bufs=2) as mz_pool,
                tc.tile_pool(name="osb", bufs=1) as osb_pool,
            ):
                y3t = zt_pool.tile([128, NPAD], F16, tag="zt", name="y3t")
                nc.sync.dma_start_transpose(y3t[:], zscr[2][:])
                out_sb = osb_pool.tile([OUT, NPAD], F32, name="out_sb")

                CW = 512
                for r0 in range(0, NPAD, CW):
                    cw = min(CW, NPAD - r0)
                    rs = slice(r0, r0 + cw)
                    p1a = psm_pool.tile([128, CW], F32, tag="ps")
                    nc.tensor.matmul(p1a[:, 0:cw], C["lw1a"][:],
                                     y3t[0:HC, rs], start=True, stop=True)
                    z1a = mz_pool.tile([128, CW], F16, tag="z1a")
                    nc.scalar.activation(z1a[:, 0:cw], p1a[:, 0:cw], Act.Relu,
                                         bias=C["lb1a"][:])
                    p1b = psm_pool.tile([128, CW], F32, tag="ps")
                    nc.tensor.matmul(p1b[:, 0:cw], C["lw1b"][:],
                                     y3t[0:HC, rs], start=True, stop=True)
                    z1b = mz_pool.tile([128, CW], F16, tag="z1b")
                    nc.scalar.activation(z1b[:, 0:cw], p1b[:, 0:cw], Act.Relu,
                                         bias=C["lb1b"][:])
                    p2 = psm_pool.tile([128, CW], F32, tag="ps")
                    nc.tensor.matmul(p2[:, 0:cw], C["lw2a"][:], z1a[:, 0:cw],
                                     start=True, stop=False)
                    nc.tensor.matmul(p2[:, 0:cw], C["lw2b"][:], z1b[:, 0:cw],
                                     start=False, stop=True)
                    z2m = mz_pool.tile([128, CW], F16, tag="z2m")
                    nc.scalar.activation(z2m[:, 0:cw], p2[:, 0:cw], Act.Relu,
                                         bias=C["lb2"][:])
                    p3 = psm_pool.tile([FC3, CW], F32, tag="ps")
                    nc.tensor.matmul(p3[:, 0:cw], C["lw3"][:], z2m[:, 0:cw],
                                     start=True, stop=True)
                    z3m = mz_pool.tile([FC3, CW], F16, tag="z3m")
                    nc.scalar.activation(z3m[:, 0:cw], p3[:, 0:cw], Act.Relu,
                                         bias=C["lb3"][:])
                    p4 = psm_pool.tile([OUT, CW], F32, tag="ps")
                    nc.tensor.matmul(p4[:, 0:cw], C["lw4"][:], z3m[:, 0:cw],
                                     start=True, stop=True)
                    nc.vector.tensor_scalar(
                        out_sb[:, rs], p4[:, 0:cw], C["lb4"][:], None, Alu.add)

                nc.sync.dma_start(P["out"][:], out_sb[:])

    nc.compile()
    return nc


_CACHE = {}


def kernel(**inputs):
    structure, in_maps = _preprocess(inputs)
    key = (tuple(structure["Tlo"]), tuple(structure["Thi"]))
    if key not in _CACHE:
        _CACHE[key] = _build(structure)
    nc = _CACHE[key]
    res = run_bass_kernel_spmd(nc, in_maps, core_ids=list(range(NCORES)))
    out = np.empty((N, OUT), np.float32)
    for c in range(NCORES):
        out[c * NPER:(c + 1) * NPER] = res.results[c]["out"].T[:NPER]
    return out
